# revision 1
# baseline (speedup 1.0000x reference)
"""Trainium2 Bass kernel for DeepSelfAttention (N=8192, D=1024) on 8 NeuronCores.

Strategy (row-parallel attention):
  - Shard the N=8192 rows of x across 8 cores (1024 rows each); replicate weights.
  - Each core computes Q/K/V projections for its row shard in feature-major
    layout (contraction dim on SBUF partitions); all operand transposes are
    done on the TensorEngine (fp32 transpose + fused fp16 cast on the
    PSUM->SBUF copy).
  - K^T and V shards are AllGathered across the 8 cores in two fp16 chunks
    (k-halves), concatenated per chunk into one flat collective, so attention
    on chunk 0 overlaps the second AllGather; Q projection and the MLP weight
    transposes fill the first AllGather's latency.
  - Flash-style one-pass attention: scores^T tiles [k=128, q=512] accumulate
    over feature tiles in PSUM, exp on ScalarE (scale=1/32 fused; scores for
    this model are provably in [-3, 3] so no max-subtraction is needed),
    A@V accumulated per (chunk, block) in PSUM with one bank-group at a time
    (PSUM start=True clears has_written for the whole bank) and flushed to an
    SBUF fp32 accumulator; softmax denominator via a ones-vector matmul.
  - The V bias is folded into the post-softmax normalize (softmax rows sum
    to 1), where it is a per-partition bias.
  - 3-layer MLP + final projection, feature-major.
All matmul operands are fp16 (full PE rate on TRN2) with fp32 PSUM
accumulation; end-to-end max rel err vs the fp32 reference is ~4e-4.
"""

import os

import numpy as np

import concourse.mybir as mybir
import concourse.tile as tile
from concourse import bacc
from concourse import bass_utils
from concourse.masks import make_identity

P = 128
D = 1024
N = 8192
NCORES = 8
NS = N // NCORES          # 1024 rows per core
DT = D // P               # 8 feature tiles
QG = 4                    # attention q groups per core
QGS = NS // QG            # 256
KB = 8                    # k blocks (one per source core)
KTB = NS // P             # 8 k tiles per block
KTH = KTB // 2            # 4 k tiles per chunk-block
CH = NS // 2              # 512 keys per chunk
KSZ = D * CH              # K-chunk elements in the flat collective buffer
VSZ = CH * D
F16 = mybir.dt.float16
F32 = mybir.dt.float32
AF = mybir.ActivationFunctionType
ALU = mybir.AluOpType

SCALE = 1.0 / np.sqrt(np.float32(D)).astype(np.float32)  # 0.03125

_CACHE = {}


def _transpose_pe(nc, raw_pool, ptr_pool, ident, src_ap, dst_tile):
    """src_ap: DRAM fp32 [R, C] -> dst_tile: SBUF fp16 [P, C//P, R] = src.T,
    via TensorEngine transpose (fp32) + ScalarE PSUM->SBUF copy w/ fp16 cast."""
    R, C = src_ap.shape
    for i in range(R // P):
        r = raw_pool.tile([P, C], F32, tag="raw")
        nc.sync.dma_start(r[:], src_ap[i * P:(i + 1) * P, :])
        for j in range(C // P):
            pst = ptr_pool.tile([P, P], F32, tag="ptr")
            nc.tensor.transpose(pst[:], r[:, j * P:(j + 1) * P], ident[:])
            nc.vector.tensor_copy(dst_tile[:, j, i * P:(i + 1) * P], pst[:])


def _build():
    nc = bacc.Bacc("TRN2", target_bir_lowering=False, debug=False,
                   num_devices=NCORES)
    xs = nc.dram_tensor("xs", [NS, D], F32, kind="ExternalInput").ap()
    W = {}
    for w in ("wq", "wk", "wv", "w1", "w2", "w3"):
        W[w] = nc.dram_tensor(w, [D, D], F32, kind="ExternalInput").ap()
    B = {}
    for b in ("bq", "bk", "bv", "b1", "b2", "b3"):
        B[b] = nc.dram_tensor(b, [D], F32, kind="ExternalInput").ap()
    fw = nc.dram_tensor("fw", [D], F32, kind="ExternalInput").ap()
    out = nc.dram_tensor("out", [1, NS], F32, kind="ExternalOutput").ap()
    debug = bool(os.environ.get("K_DEBUG"))
    dbg = {}
    if debug:
        for nm, shp, dt_ in (("dq", [D, NS], F16), ("drs", [1, NS], F32),
                             ("datt", [D, NS], F16), ("dy1", [D, NS], F16)):
            dbg[nm] = nc.dram_tensor(nm, shp, dt_, kind="ExternalOutput").ap()

    with tile.TileContext(nc) as tc:
        with (
            tc.tile_pool(name="persist", bufs=1) as pers,
            tc.tile_pool(name="dram", bufs=1, space="DRAM") as dram,
        ):
            # ---- persistent SBUF tiles ----
            qt = pers.tile([P, DT, NS], F16, tag="qt")          # Q^T
            wT = {w: pers.tile([P, DT, D], F16, tag=f"{w}T", name=f"{w}T")
                  for w in ("w1", "w2", "w3")}
            bsb = {b: pers.tile([P, DT], F32, tag=f"{b}sb", name=f"{b}sb")
                   for b in B}
            fwh = pers.tile([P, DT], F16, tag="fwh")
            ones_h = pers.tile([P, 1], F16, tag="ones")
            ones_row = pers.tile([1, P], F32, tag="ones_row")
            ident = pers.tile([P, P], F32, tag="ident")
            rs = pers.tile([1, NS], F32, tag="rs")              # softmax denom

            # ---- DRAM scratch: flat (K-chunk | V-chunk) collective buffers
            kv_d = [dram.tile([KSZ + VSZ], F16, name=f"kv_d{c}")
                    for c in range(2)]
            kvag = [dram.tile([NCORES * (KSZ + VSZ)], F16, name=f"kvag{c}",
                              addr_space="Shared")
                    for c in range(2)]

            # ---- constants ----
            for b in B:
                nc.sync.dma_start(bsb[b][:], B[b].rearrange("(t p) -> p t", p=P))
            fwf = pers.tile([P, DT], F32, tag="fwf")
            nc.sync.dma_start(fwf[:], fw.rearrange("(t p) -> p t", p=P))
            nc.vector.tensor_copy(fwh[:], fwf[:])
            nc.gpsimd.memset(ones_h[:], 1.0)
            nc.gpsimd.memset(ones_row[:], 1.0)
            make_identity(nc, ident[:])

            # ---- early pool: dies after projections ----
            early = tc.alloc_tile_pool(name="early", bufs=1)
            xsT = early.tile([P, DT, NS], F16, tag="xsT")
            for w in ("wq", "wk", "wv"):
                wT[w] = early.tile([P, DT, D], F16, tag=f"{w}T", name=f"{w}T")
            kts = early.tile([P, DT, NS], F16, tag="kts")       # K^T shard
            vs = early.tile([P, KTB, D], F16, tag="vs")         # V shard

            with (
                tc.tile_pool(name="raw", bufs=3) as raw,
                tc.tile_pool(name="ppj", bufs=4, space="PSUM") as ppj,
            ):
                # transposes on PE: x, then K/V weights (gate the AllGather),
                # then Q's
                _transpose_pe(nc, raw, ppj, ident, xs, xsT)
                for w in ("wk", "wv"):
                    _transpose_pe(nc, raw, ppj, ident, W[w], wT[w])

                # K^T = Wk @ xs^T + bk; emit + ship per k-half
                for h in range(2):
                    for dt in range(DT):
                        ps = ppj.tile([P, 512], F32, tag="ppj")
                        for et in range(DT):
                            nc.tensor.matmul(
                                ps[:],
                                wT["wk"][:, et, dt * P:(dt + 1) * P],
                                xsT[:, et, h * 512:(h + 1) * 512],
                                start=(et == 0), stop=(et == DT - 1))
                        nc.vector.tensor_tensor(
                            kts[:, dt, h * 512:(h + 1) * 512], ps[:],
                            bsb["bk"][:, dt:dt + 1].to_broadcast([P, 512]),
                            ALU.add)
                    nc.sync.dma_start(
                        kv_d[h][0:KSZ].rearrange("(t p k) -> p t k", p=P, k=CH),
                        kts[:, :, h * CH:(h + 1) * CH])
                # V = xs @ Wv.T (bias folded into post-softmax normalize)
                for h in range(2):
                    for kt in range(h * KTH, (h + 1) * KTH):
                        for dh in range(2):
                            ps = ppj.tile([P, 512], F32, tag="ppj")
                            for et in range(DT):
                                nc.tensor.matmul(
                                    ps[:],
                                    xsT[:, et, kt * P:(kt + 1) * P],
                                    wT["wv"][:, et, dh * 512:(dh + 1) * 512],
                                    start=(et == 0), stop=(et == DT - 1))
                            nc.vector.tensor_copy(
                                vs[:, kt, dh * 512:(dh + 1) * 512], ps[:])
                    nc.sync.dma_start(
                        kv_d[h][KSZ:].rearrange("(t p d) -> p t d", p=P, d=D),
                        vs[:, h * KTH:(h + 1) * KTH, :])
                    nc.gpsimd.collective_compute(
                        "AllGather", ALU.bypass,
                        replica_groups=[list(range(NCORES))],
                        ins=[kv_d[h].opt()], outs=[kvag[h].opt()])

                # work that fills the first AllGather's latency:
                # Q^T projection + MLP weight transposes
                _transpose_pe(nc, raw, ppj, ident, W["wq"], wT["wq"])
                for dt in range(DT):
                    for h in range(2):
                        ps = ppj.tile([P, 512], F32, tag="ppj")
                        for et in range(DT):
                            nc.tensor.matmul(
                                ps[:],
                                wT["wq"][:, et, dt * P:(dt + 1) * P],
                                xsT[:, et, h * 512:(h + 1) * 512],
                                start=(et == 0), stop=(et == DT - 1))
                        nc.vector.tensor_tensor(
                            qt[:, dt, h * 512:(h + 1) * 512], ps[:],
                            bsb["bq"][:, dt:dt + 1].to_broadcast([P, 512]),
                            ALU.add)
                for w in ("w1", "w2", "w3"):
                    _transpose_pe(nc, raw, ppj, ident, W[w], wT[w])

            early.release()

            if debug:
                nc.sync.dma_start(dbg["dq"].rearrange("(t p) k -> p t k", p=P),
                                  qt[:])

            # ---- attention over 2 chunks x 8 blocks ----
            pacc = tc.alloc_tile_pool(name="pacc", bufs=1)
            attacc = pacc.tile([P, DT, NS], F32, tag="attacc")
            with (
                tc.tile_pool(name="kv", bufs=3) as kv,
                tc.tile_pool(name="ex", bufs=8) as exp_pool,
                tc.tile_pool(name="psc", bufs=2, space="PSUM") as psc,
                tc.tile_pool(name="pat", bufs=4, space="PSUM") as pat,
                tc.tile_pool(name="prs", bufs=2, space="PSUM") as prs,
            ):
                for ch in range(2):
                    base = kvag[ch]
                    for kb in range(KB):
                        off = kb * (KSZ + VSZ)
                        ktb = kv.tile([P, DT, CH], F16, tag="ktb")
                        vb = kv.tile([P, KTH, D], F16, tag="vb")
                        nc.sync.dma_start(
                            ktb[:],
                            base[off:off + KSZ].rearrange(
                                "(t p k) -> p t k", p=P, k=CH))
                        nc.sync.dma_start(
                            vb[:],
                            base[off + KSZ:off + KSZ + VSZ].rearrange(
                                "(t p d) -> p t d", p=P, d=D))
                        first_blk = ch == 0 and kb == 0
                        for qp in range(2):
                            qpsl = slice(qp * 512, (qp + 1) * 512)
                            rs_ps = prs.tile([1, 512], F32, tag="prs")
                            exs = []
                            for kt in range(KTH):
                                sc = psc.tile([P, 512], F32, tag="psc")
                                for dt in range(DT):
                                    nc.tensor.matmul(
                                        sc[:],
                                        ktb[:, dt, kt * P:(kt + 1) * P],
                                        qt[:, dt, qpsl],
                                        start=(dt == 0), stop=(dt == DT - 1))
                                ex = exp_pool.tile([P, 512], F16, tag="ex",
                                                   name=f"ex{kt}")
                                nc.scalar.activation(ex[:], sc[:], AF.Exp,
                                                     scale=float(SCALE))
                                nc.tensor.matmul(rs_ps[:], ones_h[:], ex[:],
                                                 start=(kt == 0),
                                                 stop=(kt == KTH - 1),
                                                 skip_group_check=True)
                                exs.append(ex)
                            if first_blk:
                                nc.vector.tensor_copy(rs[0:1, qpsl], rs_ps[:])
                            else:
                                nc.vector.tensor_tensor(
                                    rs[0:1, qpsl], rs_ps[:], rs[0:1, qpsl],
                                    ALU.add)
                            # A@V, one PSUM-bank accumulation group at a time
                            # (start=True clears has_written bank-wide)
                            for hq in range(2):
                                qsl = slice(qp * 512 + hq * QGS,
                                            qp * 512 + (hq + 1) * QGS)
                                att_ps = [pat.tile([P, 2, QGS], F32, tag="pat",
                                                   name=f"att_ps{_j}")
                                          for _j in range(4)]
                                for dt in range(DT):
                                    for kt in range(KTH):
                                        nc.tensor.matmul(
                                            att_ps[dt // 2][:, dt % 2, :],
                                            vb[:, kt, dt * P:(dt + 1) * P],
                                            exs[kt][:, hq * QGS:(hq + 1) * QGS],
                                            start=(kt == 0),
                                            stop=(kt == KTH - 1),
                                            skip_group_check=True)
                                for j in range(4):
                                    dsl = (slice(None), slice(2 * j, 2 * j + 2),
                                           qsl)
                                    if first_blk:
                                        nc.vector.tensor_copy(attacc[dsl],
                                                              att_ps[j][:])
                                    else:
                                        nc.vector.tensor_tensor(
                                            attacc[dsl], att_ps[j][:],
                                            attacc[dsl], ALU.add)

            # ---- normalize + MLP + final ----
            with (
                tc.tile_pool(name="acts", bufs=2) as acts,
                tc.tile_pool(name="pml", bufs=4, space="PSUM") as pml,
            ):
                recip = acts.tile([1, NS], F32, tag="recip")
                out_sb = acts.tile([1, NS], F32, tag="out_sb")
                nc.vector.reciprocal(recip[:], rs[:])
                attn_h = acts.tile([P, DT, NS], F16, tag="y")
                for h in range(2):
                    qsl = slice(h * 512, (h + 1) * 512)
                    rb = pml.tile([P, 512], F32, tag="pml")
                    nc.tensor.matmul(rb[:], ones_row[:], recip[0:1, qsl])
                    for dt in range(DT):
                        nc.vector.tensor_tensor(
                            attn_h[:, dt, qsl], attacc[:, dt, qsl], rb[:],
                            ALU.mult)
                        nc.vector.tensor_tensor(
                            attn_h[:, dt, qsl], attn_h[:, dt, qsl],
                            bsb["bv"][:, dt:dt + 1].to_broadcast([P, 512]),
                            ALU.add)
                if debug:
                    nc.sync.dma_start(dbg["drs"][:], rs[:])
                    nc.sync.dma_start(
                        dbg["datt"].rearrange("(t p) q -> p t q", p=P),
                        attn_h[:])
                cur = attn_h
                for wname, bname in (("w1", "b1"), ("w2", "b2"), ("w3", "b3")):
                    nxt = acts.tile([P, DT, NS], F16, tag="y")
                    for ft in range(DT):
                        for h in range(2):
                            ps = pml.tile([P, 512], F32, tag="pml")
                            for dt in range(DT):
                                nc.tensor.matmul(
                                    ps[:],
                                    wT[wname][:, dt, ft * P:(ft + 1) * P],
                                    cur[:, dt, h * 512:(h + 1) * 512],
                                    start=(dt == 0), stop=(dt == DT - 1))
                            nc.scalar.activation(
                                nxt[:, ft, h * 512:(h + 1) * 512], ps[:],
                                AF.Relu, bias=bsb[bname][:, ft:ft + 1])
                    if debug and wname == "w1":
                        nc.sync.dma_start(
                            dbg["dy1"].rearrange("(t p) q -> p t q", p=P),
                            nxt[:])
                    cur = nxt
                for h in range(2):
                    ps = pml.tile([1, 512], F32, tag="pfin")
                    for ft in range(DT):
                        nc.tensor.matmul(
                            ps[:], fwh[:, ft:ft + 1],
                            cur[:, ft, h * 512:(h + 1) * 512],
                            start=(ft == 0), stop=(ft == DT - 1))
                    nc.vector.tensor_copy(out_sb[0:1, h * 512:(h + 1) * 512],
                                          ps[:])
                nc.sync.dma_start(out[:], out_sb[:])
            pacc.release()

    nc.compile()
    return nc


def _get_nc():
    if "nc" not in _CACHE:
        _CACHE["nc"] = _build()
    return _CACHE["nc"]


def kernel(**inputs):
    nc = _get_nc()
    x = np.ascontiguousarray(np.asarray(inputs["x"], dtype=np.float32))
    names = {"wq": "Wq", "wk": "Wk", "wv": "Wv", "w1": "W1", "w2": "W2",
             "w3": "W3", "bq": "bq", "bk": "bk", "bv": "bv", "b1": "b1",
             "b2": "b2", "b3": "b3"}
    shared = {k: np.ascontiguousarray(np.asarray(inputs[v], dtype=np.float32))
              for k, v in names.items()}
    shared["fw"] = np.ascontiguousarray(
        np.asarray(inputs["final_weight"], dtype=np.float32).reshape(D))
    in_maps = []
    for c in range(NCORES):
        m = dict(shared)
        m["xs"] = np.ascontiguousarray(x[c * NS:(c + 1) * NS, :])
        in_maps.append(m)
    res = bass_utils.run_bass_kernel_spmd(
        nc, in_maps, core_ids=list(range(NCORES)))
    if os.environ.get("K_DEBUG"):
        kernel.debug_results = res.results
    return np.concatenate(
        [res.results[c]["out"].reshape(NS) for c in range(NCORES)])



# revision 12
# speedup vs baseline: 1.9658x; 1.9658x over previous
"""Trainium2 Bass kernel for DeepSelfAttention (N=8192, D=1024) on 8 NeuronCores.

Row-parallel attention, fp8 (e4m3) DoubleRow matmuls everywhere except the
MLP (fp16), chosen to stay ~5x under the 2e-2 rel-err gate (host-simulated
max rel err ~4e-3):

  - Host pre-transposes and pre-casts operands: x^T shard and (256*W)^T for
    Wq/Wk/Wv in e4m3 (scale 256 keeps the +-1/32 weights out of e4m3's
    subnormal range; the 1/256 is folded into the PSUM->SBUF cast), W^T in
    fp16 for the MLP. bk is dropped entirely: it shifts every score in a
    softmax row by the same constant (Q[q] . bk), which softmax ignores.
  - K^T/V are computed per 256-key chunk and AllGathered in four fp8 chunks
    so the first chunk's gather lands just as the Q projection finishes; a
    zero-byte dummy AllGather issued at kernel start absorbs the one-time
    ~47us comm-init barrier. The program is core-symmetric: every core
    processes all 8 gathered key blocks, its own included.
  - Attention: DoubleRow fp8 matmuls contract 256 elements per pass. Per
    256-query group, scores -> exp (ScalarE, scale=1/32 fused; scores are
    ~N(0,0.33) so exp stays far below e4m3's 240 max) -> A@V accumulated
    over a whole chunk's 2048 keys in one 4-bank PSUM group, flushed to an
    SBUF fp32 accumulator once per (chunk, group). Softmax denominators via
    a DoubleRow ones-vector matmul. V's bias is folded into the
    post-softmax normalize (softmax rows sum to 1).
  - 3-layer MLP + final projection in fp16 (fp8 MLP fails the error gate).
"""

import os

import ml_dtypes
import numpy as np

import concourse.mybir as mybir
import concourse.tile as tile
from concourse import bacc
from concourse import bass_utils

P = 128
D = 1024
N = 8192
NCORES = 8
NS = N // NCORES          # 1024 rows per core
DT = D // P               # 8 feature tiles
ETP = DT // 2             # 4 feature-pair tiles (DoubleRow)
QG = 4                    # query groups per core
QGS = NS // QG            # 256
NCH = 4                   # collective chunks
CH = NS // NCH            # 256 keys per chunk
KSZ = DT * P * CH         # K-chunk bytes (e4m3) in the flat collective buffer
VSZ = 2 * P * D           # V-chunk bytes
F8 = mybir.dt.float8e4
F16 = mybir.dt.float16
F32 = mybir.dt.float32
AF = mybir.ActivationFunctionType
ALU = mybir.AluOpType
DR = mybir.MatmulPerfMode.DoubleRow

SCALE = 1.0 / 32.0        # 1/sqrt(D), exact
WS = 256.0                # host-side QKV weight pre-scale (exact power of 2)

_CACHE = {}


def _build(debug):
    nc = bacc.Bacc("TRN2", target_bir_lowering=False, debug=False,
                   num_devices=NCORES)
    xt = nc.dram_tensor("xt", [D, NS], F8, kind="ExternalInput").ap()
    w8 = {w: nc.dram_tensor(w, [D, D], F8, kind="ExternalInput").ap()
          for w in ("wqt", "wkt", "wvt")}
    w16 = {w: nc.dram_tensor(w, [D, D], F16, kind="ExternalInput").ap()
           for w in ("w1t", "w2t", "w3t")}
    B = {b: nc.dram_tensor(b, [D], F32, kind="ExternalInput").ap()
         for b in ("bq", "bv", "b1", "b2", "b3")}
    fw = nc.dram_tensor("fw", [D], F16, kind="ExternalInput").ap()
    out = nc.dram_tensor("out", [1, NS], F32, kind="ExternalOutput").ap()
    dbg = {}
    if debug:
        for nm, shp, dt_ in (("dq", [D, NS], F8), ("drs", [1, NS], F32),
                             ("datt", [D, NS], F32)):
            dbg[nm] = nc.dram_tensor(nm, shp, dt_, kind="ExternalOutput").ap()

    with tile.TileContext(nc) as tc:
        with (
            tc.tile_pool(name="pers", bufs=1) as pers,
            tc.tile_pool(name="dram", bufs=1, space="DRAM") as dram,
        ):
            # stack order (LIFO release): patp/mw1 below kv below early
            patp = tc.alloc_tile_pool(name="pat_sb", bufs=1)
            mw1 = tc.alloc_tile_pool(name="mw1", bufs=1)
            # K/V tiles cycle per-block buffers: chunk ch+2 reuses chunk ch's
            # slot once phase ch's attention has consumed it
            kvp = tc.alloc_tile_pool(name="kv", bufs=2)
            # dummy collective first: absorbs the one-time comm-init barrier
            ini_d = dram.tile([256], F8, name="ini_d")
            ini_g = dram.tile([NCORES * 256], F8, name="ini_g",
                              addr_space="Shared")
            nc.gpsimd.collective_compute(
                "AllGather", ALU.bypass,
                replica_groups=[list(range(NCORES))],
                ins=[ini_d.opt()], outs=[ini_g.opt()])

            # ---- persistent SBUF ----
            qt8 = pers.tile([P, DT, NS], F8, tag="qt8")
            rs = pers.tile([1, NS], F32, tag="rs")
            recip = pers.tile([1, NS], F32, tag="recip")
            bsb = {b: pers.tile([P, DT], F32, tag=f"{b}sb", name=f"{b}sb")
                   for b in B}
            fwh = pers.tile([P, DT], F16, tag="fwh")
            ones8 = pers.tile([P, 2, 16], F8, tag="ones8")
            onesf = pers.tile([P, 2, 1], F32, tag="onesf")
            ones_row = pers.tile([1, P], F32, tag="ones_row")

            # K^T / V gathered tiles, created per (block, chunk) at DMA time
            ktb = [[None] * NCH for _ in range(NCORES)]
            vb = [[None] * NCH for _ in range(NCORES)]

            kv_d = [dram.tile([KSZ + VSZ], F8, name=f"kv_d{c}")
                    for c in range(NCH)]
            kvag = [dram.tile([NCORES * (KSZ + VSZ)], F8, name=f"kvag{c}",
                              addr_space="Shared")
                    for c in range(NCH)]

            # ---- constants ----
            for b in B:
                nc.sync.dma_start(bsb[b][:], B[b].rearrange("(t p) -> p t", p=P))
            nc.sync.dma_start(fwh[:], fw.rearrange("(t p) -> p t", p=P))
            nc.gpsimd.memset(onesf[:], 1.0)
            nc.gpsimd.memset(ones_row[:], 1.0)
            nc.vector.tensor_copy(ones8[:, :, 0:1], onesf[:])

            # ---- projections (all DoubleRow fp8) ----
            early = tc.alloc_tile_pool(name="early", bufs=1)
            xsT = early.tile([P, DT, NS], F8, tag="xsT")
            wsb = {w: early.tile([P, DT, D], F8, tag=w, name=w)
                   for w in ("wkt", "wvt", "wqt")}
            nc.sync.dma_start(wsb["wkt"][:],
                              w8["wkt"].rearrange("(t p) f -> p t f", p=P))
            nc.sync.dma_start(xsT[:], xt.rearrange("(t p) n -> p t n", p=P))
            nc.sync.dma_start(wsb["wvt"][:],
                              w8["wvt"].rearrange("(t p) f -> p t f", p=P))
            nc.sync.dma_start(wsb["wqt"][:],
                              w8["wqt"].rearrange("(t p) f -> p t f", p=P))

            with (
                tc.tile_pool(name="stage", bufs=2) as stage,
                tc.tile_pool(name="ppj", bufs=4, space="PSUM") as ppj,
            ):
                for ch in range(NCH):
                    csl = slice(ch * CH, (ch + 1) * CH)
                    # K^T chunk: [1024 features, 256 keys]
                    kst = stage.tile([P, DT, CH], F8, tag="kst")
                    for dt in range(DT):
                        ps = ppj.tile([P, CH], F32, tag="ppj")
                        for et in range(ETP):
                            nc.tensor.matmul(
                                ps[:],
                                wsb["wkt"][:, 2 * et:2 * et + 2,
                                           dt * P:(dt + 1) * P],
                                xsT[:, 2 * et:2 * et + 2, csl],
                                start=(et == 0), stop=(et == ETP - 1),
                                perf_mode=DR)
                        nc.scalar.activation(kst[:, dt, :], ps[:],
                                             AF.Copy, scale=1.0 / WS)
                    nc.sync.dma_start(
                        kv_d[ch][0:KSZ].rearrange("(t p k) -> p t k",
                                                  p=P, k=CH),
                        kst[:])
                    # V chunk: [256 keys, 1024 features]
                    vst = stage.tile([P, 2, D], F8, tag="vst")
                    for kt in range(2):
                        nsl = slice((ch * 2 + kt) * P, (ch * 2 + kt + 1) * P)
                        for dh in range(2):
                            ps = ppj.tile([P, 512], F32, tag="ppjv")
                            for et in range(ETP):
                                nc.tensor.matmul(
                                    ps[:],
                                    xsT[:, 2 * et:2 * et + 2, nsl],
                                    wsb["wvt"][:, 2 * et:2 * et + 2,
                                               dh * 512:(dh + 1) * 512],
                                    start=(et == 0), stop=(et == ETP - 1),
                                    perf_mode=DR)
                            nc.scalar.activation(
                                vst[:, kt, dh * 512:(dh + 1) * 512],
                                ps[:], AF.Copy, scale=1.0 / WS)
                    nc.sync.dma_start(
                        kv_d[ch][KSZ:].rearrange("(t p d) -> p t d",
                                                 p=P, d=D),
                        vst[:])
                    nc.gpsimd.collective_compute(
                        "AllGather", ALU.bypass,
                        replica_groups=[list(range(NCORES))],
                        ins=[kv_d[ch].opt()], outs=[kvag[ch].opt()])
                    # gathered K/V loads (wait on the collective, run during
                    # earlier chunks' attention)
                    for b in range(NCORES):
                        off = b * (KSZ + VSZ)
                        ktb[b][ch] = kvp.tile([P, DT, CH], F8, tag=f"ktb{b}",
                                              name=f"ktb{b}_{ch}")
                        vb[b][ch] = kvp.tile([P, 2, D], F8, tag=f"vb{b}",
                                             name=f"vb{b}_{ch}")
                        nc.sync.dma_start(
                            ktb[b][ch][:],
                            kvag[ch][off:off + KSZ].rearrange(
                                "(t p k) -> p t k", p=P, k=CH))
                        nc.sync.dma_start(
                            vb[b][ch][:],
                            kvag[ch][off + KSZ:off + KSZ + VSZ].rearrange(
                                "(t p d) -> p t d", p=P, d=D))

                # Q^T projection (bias fused into the fp8 cast)
                for dt in range(DT):
                    for h in range(2):
                        ps = ppj.tile([P, 512], F32, tag="ppjv")
                        for et in range(ETP):
                            nc.tensor.matmul(
                                ps[:],
                                wsb["wqt"][:, 2 * et:2 * et + 2,
                                           dt * P:(dt + 1) * P],
                                xsT[:, 2 * et:2 * et + 2,
                                    h * 512:(h + 1) * 512],
                                start=(et == 0), stop=(et == ETP - 1),
                                perf_mode=DR)
                        nc.scalar.activation(
                            qt8[:, dt, h * 512:(h + 1) * 512], ps[:],
                            AF.Identity, scale=1.0 / WS,
                            bias=bsb["bq"][:, dt:dt + 1])

            early.release()

            if debug:
                nc.sync.dma_start(
                    dbg["dq"].rearrange("(t p) n -> p t n", p=P), qt8[:])

            # ---- attention: 4 chunk-phases x 4 query-groups; per (phase,
            # group) one PSUM accumulation over the chunk's 2048 keys ----
            attacc = patp.tile([P, DT, NS], F32, tag="attacc")
            w1T = mw1.tile([P, DT, D], F16, tag="w1T")
            nc.sync.dma_start(w1T[:],
                              w16["w1t"].rearrange("(t p) f -> p t f", p=P))

            with (
                tc.tile_pool(name="ex", bufs=4) as exp_pool,
                tc.tile_pool(name="psc", bufs=2, space="PSUM") as psc,
                tc.tile_pool(name="pat", bufs=1, space="PSUM") as pat,
                tc.tile_pool(name="prs", bufs=2, space="PSUM") as prs,
            ):
                for ch in range(NCH):
                    first = ch == 0
                    n = NCORES
                    for qg in range(QG):
                        qsl = slice(qg * QGS, (qg + 1) * QGS)
                        rs_ps = prs.tile([1, QGS], F32, tag="prs")
                        pat_t = pat.tile([P, DT, QGS], F32, tag="pat")

                        def scores(b):
                            ps = psc.tile([P, 2, QGS], F32, tag="psc")
                            for j in range(2):
                                for et in range(ETP):
                                    nc.tensor.matmul(
                                        ps[:, j, :],
                                        ktb[b][ch][:, 2 * et:2 * et + 2,
                                                   j * P:(j + 1) * P],
                                        qt8[:, 2 * et:2 * et + 2, qsl],
                                        start=(et == 0), stop=(et == ETP - 1),
                                        perf_mode=DR, skip_group_check=True)
                            ex = exp_pool.tile([P, 2, QGS], F8, tag="ex")
                            nc.scalar.activation(ex[:], ps[:], AF.Exp,
                                                 scale=SCALE)
                            return ex

                        def av(b, ex):
                            for dt in range(DT):
                                nc.tensor.matmul(
                                    pat_t[:, dt, :],
                                    vb[b][ch][:, :, dt * P:(dt + 1) * P],
                                    ex[:],
                                    start=(b == 0), stop=(b == n - 1),
                                    perf_mode=DR, skip_group_check=True)
                            nc.tensor.matmul(
                                rs_ps[:], ones8[:, :, 0:1], ex[:],
                                start=(b == 0), stop=(b == n - 1),
                                perf_mode=DR, skip_group_check=True)

                        prev = None
                        for b in range(n):
                            ex = scores(b)
                            if prev is not None:
                                av(b - 1, prev)
                            prev = ex
                        av(n - 1, prev)
                        if first:
                            nc.vector.tensor_copy(attacc[:, :, qsl], pat_t[:])
                            nc.vector.tensor_copy(rs[0:1, qsl], rs_ps[:])
                        else:
                            nc.vector.tensor_tensor(
                                attacc[:, :, qsl], pat_t[:],
                                attacc[:, :, qsl], ALU.add)
                            nc.vector.tensor_tensor(
                                rs[0:1, qsl], rs_ps[:], rs[0:1, qsl],
                                ALU.add)

            kvp.release()

            if debug:
                nc.sync.dma_start(dbg["drs"][:], rs[:])
                nc.sync.dma_start(
                    dbg["datt"].rearrange("(t p) n -> p t n", p=P), attacc[:])

            # ---- normalize + MLP (fp16) + final ----
            mw2 = tc.alloc_tile_pool(name="mw2", bufs=1)
            wT = {"w1": w1T}
            for w in ("w2", "w3"):
                wT[w] = mw2.tile([P, DT, D], F16, tag=f"{w}T", name=f"{w}T")
                nc.sync.dma_start(
                    wT[w][:], w16[f"{w}t"].rearrange("(t p) f -> p t f", p=P))
            with (
                tc.tile_pool(name="acts", bufs=2) as acts,
                tc.tile_pool(name="pml", bufs=2, space="PSUM") as pml,
                tc.tile_pool(name="prb", bufs=2, space="PSUM") as prb,
            ):
                out_sb = acts.tile([1, NS], F32, tag="out_sb")
                nc.vector.reciprocal(recip[:], rs[:])
                attn_h = acts.tile([P, DT, NS], F16, tag="y")
                for h in range(2):
                    qsl = slice(h * 512, (h + 1) * 512)
                    rb = prb.tile([P, 512], F32, tag="rb")
                    nc.tensor.matmul(rb[:], ones_row[:], recip[0:1, qsl])
                    for dt in range(DT):
                        nc.vector.tensor_tensor(
                            attn_h[:, dt, qsl], attacc[:, dt, qsl], rb[:],
                            ALU.mult)
                        nc.vector.tensor_tensor(
                            attn_h[:, dt, qsl], attn_h[:, dt, qsl],
                            bsb["bv"][:, dt:dt + 1].to_broadcast([P, 512]),
                            ALU.add)
                cur = attn_h
                for wname, bname in (("w1", "b1"), ("w2", "b2"), ("w3", "b3")):
                    nxt = acts.tile([P, DT, NS], F16, tag="y")
                    for ft in range(DT):
                        for h in range(2):
                            ps = pml.tile([P, 512], F32, tag="pml")
                            for dt in range(DT):
                                nc.tensor.matmul(
                                    ps[:],
                                    wT[wname][:, dt, ft * P:(ft + 1) * P],
                                    cur[:, dt, h * 512:(h + 1) * 512],
                                    start=(dt == 0), stop=(dt == DT - 1))
                            nc.scalar.activation(
                                nxt[:, ft, h * 512:(h + 1) * 512], ps[:],
                                AF.Relu, bias=bsb[bname][:, ft:ft + 1])
                    cur = nxt
                for h in range(2):
                    ps = pml.tile([1, 512], F32, tag="pfin")
                    for ft in range(DT):
                        nc.tensor.matmul(
                            ps[:], fwh[:, ft:ft + 1],
                            cur[:, ft, h * 512:(h + 1) * 512],
                            start=(ft == 0), stop=(ft == DT - 1))
                    nc.vector.tensor_copy(out_sb[0:1, h * 512:(h + 1) * 512],
                                          ps[:])
                nc.sync.dma_start(out[:], out_sb[:])
            mw2.release()
            mw1.release()
            patp.release()

    nc.compile()
    return nc


def _get_nc():
    debug = bool(os.environ.get("K_DEBUG"))
    key = ("nc", debug)
    if key not in _CACHE:
        _CACHE[key] = _build(debug)
    return _CACHE[key]


def make_in_maps(inputs):
    E4 = ml_dtypes.float8_e4m3
    f32 = lambda a: np.asarray(a, dtype=np.float32)
    x = f32(inputs["x"])
    shared = {}
    for nm, w in (("wqt", "Wq"), ("wkt", "Wk"), ("wvt", "Wv")):
        shared[nm] = np.ascontiguousarray(
            (f32(inputs[w]) * np.float32(WS)).T).astype(E4)
    for nm, w in (("w1t", "W1"), ("w2t", "W2"), ("w3t", "W3")):
        shared[nm] = np.ascontiguousarray(f32(inputs[w]).T).astype(np.float16)
    for b in ("bq", "bv", "b1", "b2", "b3"):
        shared[b] = np.ascontiguousarray(f32(inputs[b]))
    shared["fw"] = np.ascontiguousarray(
        f32(inputs["final_weight"]).reshape(D)).astype(np.float16)
    in_maps = []
    for c in range(NCORES):
        m = dict(shared)
        m["xt"] = np.ascontiguousarray(x[c * NS:(c + 1) * NS, :].T).astype(E4)
        in_maps.append(m)
    return in_maps


def kernel(**inputs):
    nc = _get_nc()
    in_maps = make_in_maps(inputs)
    res = bass_utils.run_bass_kernel_spmd(
        nc, in_maps, core_ids=list(range(NCORES)))
    if os.environ.get("K_DEBUG"):
        kernel.debug_results = res.results
    return np.concatenate(
        [res.results[c]["out"].reshape(NS) for c in range(NCORES)])
